# revision 37
# baseline (speedup 1.0000x reference)
"""Distributed Trainium2 Bass kernel for a causal attention block + LayerNorm.

Reference computation (B=2, T=2048, C=1024, H=16 heads, Dh=64):
    q,k,v = x@Wq+bq, x@Wk+bk, x@Wv+bv          (per-head split)
    att   = softmax(causal(q k^T / sqrt(Dh)))
    o     = att @ v ; y = o@Wo + bo ; out = LayerNorm(y) * gamma + beta

Sharding (8 cores, one TRN2 chip):
    Tensor-parallel over heads: core i owns heads {2i, 2i+1} for BOTH
    batches (Megatron-style column shards of Wq/Wk/Wv).  After attention,
    two 8-core AllToAlls (one per local head, bf16 payload) redistribute the
    per-head outputs (plus softmax denominators) to token-sharding: core i
    ends with tokens [b = i//4, t in (i%4)*512 ...] with ALL 1024 features,
    applies the softmax division, output projection (full Wo), bias and
    LayerNorm locally, and writes its (512, 1024) slice of the output.

Schedule (vs the v3 baseline, ~350us -> ~270us):
    - x^T DMA'd in 8 per-(b, q-block) slices in consumption order so the
      first projection starts ~8us in instead of waiting for the full 8.4MB;
      Wo + LN constants loaded at kernel start.
    - q/k projection accumulation chains interleaved (consecutive PE
      matmuls alternate PSUM banks; same-bank accumulation serializes the
      PE at ~2x cost) and attention emission software-pipelined: scores
      for group g+1 issued before P@V of group g.
    - softmax normalization entirely off the scalar engine: raw
      denominators ride the AllToAll (row 64); receive side does bf16->
      fp32 copy -> DVE reciprocal_approx_fast -> bf16, a partition
      broadcast (K=1 PE matmul into the idle psOC ring for hp0, gpsimd
      for hp1), multiply on DVE.  The scalar queue holds ONLY exps + the
      LayerNorm accumulations, in two stable activation tables, so no
      per-tile ACT_TABLE_LOADs and no cross-collective scalar dependency
      a hoisted schedule could head-of-line block on (v3 lost ~22us).
    - second AllToAll triggered the moment head-1 attention's payload
      DMAs land: the gpsimd queue is exactly [A2A#0][A2A#1] (collectives
      hold that queue to completion, so nothing may sit between).
    - output projection split-K: the head-0 half (K rows 0..63 of oT/Wo,
      all 4 token tiles) accumulates into PSUM while the second AllToAll
      is in flight; the head-1 half + bias + LayerNorm land after it, two
      token tiles at a time, ft-outer so consecutive matmuls alternate
      PSUM regions.  Attention PSUM pools are closed and a 4x[128,1024]
      pool opened so all 4 token tiles accumulate concurrently.
    - scheduler control: the overlap work carries tile_wait_until(0.200)
      and the second AllToAll tile_wait_until(0.210).  The list scheduler
      strengthens semaphore waits to match its simulated order, so (a)
      un-pinned overlap work gets interleaved into the attn(1,..) engine
      queues (its collective model is optimistic), and (b) work placed
      after a collective's simulated completion gets chained to that
      collective at runtime.  Pinning the trigger LATER than its overlap
      work is what keeps the overlap waits encoded against A2A#0 only;
      the pins are scheduling-time-only and cost nothing at runtime.

Layout choices (all on-chip matmuls contract over the partition axis):
    - activations are feature-major: host passes x^T [C, B, T].
    - q^T,k^T,v^T [d, t] produced directly; v transposed on the PE into
      s-major v-hat [s, d] with an extra ones column per head so the P@V
      matmul also yields the softmax denominator for free.
    - scores are computed transposed: S^T[s, q] = k^T.T @ q^T; score chunks
      are packed in pairs into 2-bank PSUM tiles so each scalar-engine Exp
      call covers up to 1024 columns; causal masking via a triangular
      bf16 multiply on the DVE for the 128-wide diagonal blocks only;
      P@V uses v-hat as the stationary operand so the unnormalized attention
      output O^T [d, q] is produced feature-major (no transposes needed).
"""

import numpy as np
import ml_dtypes

import concourse.bass as bass
import concourse.mybir as mybir
import concourse.tile as tile
from concourse import bacc
from concourse.bass_utils import run_bass_kernel_spmd
F32 = mybir.dt.float32
BF16 = mybir.dt.bfloat16
AF = mybir.ActivationFunctionType
OP = mybir.AluOpType

B, T, C, H, Dh = 2, 2048, 1024, 16, 64
NCORES = 8
HPC = 2               # heads per core
DPC = HPC * Dh        # 128 feature columns per core
TS = 512              # output token-slice length per core
NQB = T // 512        # 4 q blocks
NST = T // 128        # 16 s tiles
NCT = C // 128        # 8 contraction tiles
EPS = 1e-5

DT_X = BF16
DT_W = BF16
DT_P = BF16
DT_A2A = BF16         # AllToAll payload dtype
NP_X = ml_dtypes.bfloat16
NP_W = ml_dtypes.bfloat16

_CACHE = {}


def _build():
    nc = bacc.Bacc("TRN2", target_bir_lowering=False, debug=False,
                   num_devices=NCORES)

    xT_h = nc.dram_tensor("xT", [128, NCT, B, T], DT_X, kind="ExternalInput")
    wq_h = nc.dram_tensor("wq", [128, NCT, DPC], DT_W, kind="ExternalInput")
    wk_h = nc.dram_tensor("wk", [128, NCT, DPC], DT_W, kind="ExternalInput")
    wv_h = nc.dram_tensor("wv", [128, NCT, DPC], DT_W, kind="ExternalInput")
    wo_h = nc.dram_tensor("wo", [128, NCT, C], DT_W, kind="ExternalInput")
    bqT_h = nc.dram_tensor("bqT", [DPC, 1], F32, kind="ExternalInput")
    bkT_h = nc.dram_tensor("bkT", [DPC, 1], F32, kind="ExternalInput")
    bvT_h = nc.dram_tensor("bvT", [DPC, 1], F32, kind="ExternalInput")
    bo_h = nc.dram_tensor("bo_row", [1, C], BF16, kind="ExternalInput")
    gam_h = nc.dram_tensor("gamb", [128, C], BF16, kind="ExternalInput")
    bet_h = nc.dram_tensor("betb", [128, C], BF16, kind="ExternalInput")
    out_h = nc.dram_tensor("out", [TS, C], BF16, kind="ExternalOutput")

    ones1_d = nc.inline_tensor(np.ones((1, 128), ml_dtypes.bfloat16), name="ones1_const")
    ident_d = nc.inline_tensor(
        np.eye(128, dtype=ml_dtypes.bfloat16), name="ident_const")
    tri_np = (np.tril(np.ones((128, 128), np.float32)).T).astype(ml_dtypes.bfloat16)
    tri_d = nc.inline_tensor(tri_np, name="tri_const")

    with tile.TileContext(nc) as tc:
        with (
            tc.tile_pool(name="const", bufs=1) as cp,
            tc.tile_pool(name="dram", bufs=1, space="DRAM") as dp,
            tc.tile_pool(name="act", bufs=1) as ap,
            tc.tile_pool(name="xw", bufs=1) as xw,
            tc.tile_pool(name="wop", bufs=1) as wop,
            tc.tile_pool(name="lnp", bufs=2) as lnp,
            tc.tile_pool(name="pp", bufs=5) as pp,
            tc.tile_pool(name="vtp", bufs=2) as vtp,
            tc.tile_pool(name="ohp", bufs=3) as ohp,
            tc.tile_pool(name="orp", bufs=3) as orp,
        ):
            # attention-phase PSUM pools (closed before the out-projection,
            # which needs all 8 banks for 4 concurrent [128,1024] tiles)
            psa_ctx = [
                tc.tile_pool(name="psM", bufs=2, space="PSUM"),
                tc.tile_pool(name="psS2", bufs=2, space="PSUM"),
                tc.tile_pool(name="psOC", bufs=2, space="PSUM"),
            ]
            psM, psS2, psOC = (c.__enter__() for c in psa_ctx)
            psM, psS2, psOC = psM, psS2, psOC

            # ---- weights first (small, needed immediately); x^T in 32
            # per-(b,qb) slices on the gpsimd queue, in the exact order the
            # projections consume them ----
            wq = xw.tile([128, NCT, DPC], DT_W)
            wk = xw.tile([128, NCT, DPC], DT_W)
            wv = xw.tile([128, NCT, DPC], DT_W)
            for w_sb, w_h in ((wq, wq_h), (wk, wk_h), (wv, wv_h)):
                nc.sync.dma_start(w_sb[:], w_h[:])

            # warm-up AllToAll FIRST on the gpsimd queue: the runtime's
            # pre-first-collective barrier (40-128us, cross-core variance)
            # runs concurrently with phase 1 either way, but the warmup's
            # trigger must not queue behind the DMA-issue backlog -- on
            # slow-barrier runs that serialized straight into AllToAll#0
            warm_in = dp.tile([NCORES, 8], F32, tag="wi")
            warm_out = dp.tile([NCORES, 8], F32, tag="wo_")
            warm_sb = cp.tile([NCORES, 8], F32)
            nc.gpsimd.memset(warm_sb[:], 0.0)
            nc.sync.dma_start(warm_in[:], warm_sb[:])
            nc.gpsimd.collective_compute(
                "AllToAll", OP.bypass, replica_groups=[list(range(NCORES))],
                ins=[warm_in.opt()], outs=[warm_out.opt()])

            xT = xw.tile([128, NCT, B, T], DT_X)
            for b in range(B):
                for qb in range(NQB):
                    sl = slice(qb * 512, (qb + 1) * 512)
                    # one DMA per slice: each gpsimd dma_start issue costs
                    # ~640ns of DIRECT2D, so ct-chunking the first slice
                    # (8 issues) DELAYS its completion past a single issue
                    nc.gpsimd.dma_start(xT[:, :, b, sl],
                                        xT_h[:, :, b, sl])
            # Wo + LN constants now -- they are needed right after the last
            # collective and must not queue behind it
            wo = wop.tile([128, NCT, C], DT_W, tag="wo")
            nc.gpsimd.dma_start(wo[:], wo_h[:])
            bo = wop.tile([1, C], BF16, tag="bo")
            nc.sync.dma_start(bo[:], bo_h[:])
            gam = wop.tile([128, C], BF16, tag="gam")
            nc.sync.dma_start(gam[:], gam_h[:])
            bet = wop.tile([128, C], BF16, tag="bet")
            nc.sync.dma_start(bet[:], bet_h[:])
            eps_t = wop.tile([128, 1], F32, tag="eps")
            nc.gpsimd.memset(eps_t[:], EPS)

            # ---- small constants (issued on sync queue) ----
            bqT = cp.tile([DPC, 1], F32)
            nc.sync.dma_start(bqT[:], bqT_h[:])
            bkT = cp.tile([DPC, 1], F32)
            nc.sync.dma_start(bkT[:], bkT_h[:])
            bvT = cp.tile([DPC, 1], F32)
            nc.sync.dma_start(bvT[:], bvT_h[:])
            ident = cp.tile([128, 128], BF16)
            nc.sync.dma_start(ident[:], ident_d[:])
            ones1 = cp.tile([1, 128], BF16)
            nc.sync.dma_start(ones1[:], ones1_d[:])
            tri = cp.tile([128, 128], BF16)
            nc.sync.dma_start(tri[:], tri_d[:])

            # ---- persistent activation tiles ----
            qT = ap.tile([DPC, B, T], DT_P)
            kT = ap.tile([DPC, B, T], DT_P)
            vhat = ap.tile([128, B, NST, HPC, 65], DT_P)
            oT = ap.tile([128, NCT, 512], DT_P)
            for b in range(B):
                nc.gpsimd.memset(vhat[:, b, :, :, 64:65], 1.0)

            a2a_in0 = dp.tile([NCORES, 65, 512], DT_A2A, tag="ai0")
            a2a_in1 = dp.tile([NCORES, 65, 512], DT_A2A, tag="ai1")
            a2a_out0 = dp.tile([NCORES, 65, 512], DT_A2A, tag="ao0")
            a2a_out1 = dp.tile([NCORES, 65, 512], DT_A2A, tag="ao1")
            a2a_in = [a2a_in0, a2a_in1]
            a2a_out = [a2a_out0, a2a_out1]

            def proj(b, qb):
                sl = slice(qb * 512, (qb + 1) * 512)
                # q and k accumulation chains interleaved: consecutive PE
                # matmuls alternate PSUM banks (same-bank accumulation
                # serializes the PE at ~2x cost)
                ps_q = psM.tile([128, 512], F32, tag="m", name="ps_q")
                ps_k = psM.tile([128, 512], F32, tag="m", name="ps_k")
                for ct in range(NCT):
                    nc.tensor.matmul(ps_q[:], wq[:, ct], xT[:, ct, b, sl],
                                     start=(ct == 0), stop=(ct == NCT - 1))
                    nc.tensor.matmul(ps_k[:], wk[:, ct], xT[:, ct, b, sl],
                                     start=(ct == 0), stop=(ct == NCT - 1))
                nc.vector.tensor_scalar_add(qT[:, b, sl], ps_q[:], bqT[:])
                nc.vector.tensor_scalar_add(kT[:, b, sl], ps_k[:], bkT[:])
                # v^T, then transpose 128x128 blocks into s-major vhat
                ps = psM.tile([128, 512], F32, tag="m")
                for ct in range(NCT):
                    nc.tensor.matmul(ps[:], wv[:, ct], xT[:, ct, b, sl],
                                     start=(ct == 0), stop=(ct == NCT - 1))
                vt = vtp.tile([128, 512], DT_P, tag="vt")
                nc.vector.tensor_scalar_add(vt[:], ps[:], bvT[:])
                for sub in range(4):
                    st = qb * 4 + sub
                    tr = psM.tile([128, 128], DT_P, tag="m")
                    nc.tensor.transpose(
                        tr[:], vt[:, sub * 128:(sub + 1) * 128], ident[:])
                    nc.vector.tensor_copy(
                        vhat[:, b, st, :, 0:64],
                        tr[:].rearrange("p (hh d) -> p hh d", hh=HPC))

            def attn(hh, b, qb):
                hlo = hh * 64
                o_ps = psOC.tile([65, 512], F32, tag="o")
                nsi = 4 * qb + 4
                # chunks (si, lo): lo = in-block column offset; pack pairs
                # into one 2-bank PSUM tile so each exp covers both
                chunks = [(si, 0) for si in range(4 * qb)] + \
                         [(si, si * 128 - qb * 512) for si in range(4 * qb, nsi)]
                groups = []
                i = 0
                while i < len(chunks):
                    w0 = 512 - chunks[i][1]
                    if i + 1 < len(chunks) and w0 + (512 - chunks[i + 1][1]) <= 1024:
                        groups.append([chunks[i], chunks[i + 1]])
                        i += 2
                    else:
                        groups.append([chunks[i]])
                        i += 1

                def emit_scores(grp):
                    s_ps = psS2.tile([128, 1024], F32, tag="s2")
                    off = 0
                    for si, lo in grp:
                        w = 512 - lo
                        nc.tensor.matmul(
                            s_ps[:, off:off + w],
                            kT[hlo:hlo + 64, b, si * 128:(si + 1) * 128],
                            qT[hlo:hlo + 64, b, qb * 512 + lo:(qb + 1) * 512],
                            start=True, stop=True)
                        off += w
                    return s_ps

                # software pipeline: scores for group g+1 are on the PE queue
                # before P@V of group g, so the PE never waits out the exp
                s_cur = emit_scores(groups[0])
                for g, grp in enumerate(groups):
                    tot = sum(512 - lo for _, lo in grp)
                    p_sb = pp.tile([128, 1024], DT_P, tag="p")
                    nc.scalar.activation(p_sb[:, 0:tot], s_cur[:, 0:tot],
                                         AF.Exp, scale=0.125)
                    if g + 1 < len(groups):
                        s_cur = emit_scores(groups[g + 1])
                    off = 0
                    for si, lo in grp:
                        w = 512 - lo
                        if lo > 0 or si * 128 == qb * 512:
                            # diagonal block: causal triangle mask (on DVE --
                            # gpsimd hosts the collective triggers and would
                            # serialize behind them)
                            nc.vector.tensor_tensor(
                                p_sb[:, off:off + 128], p_sb[:, off:off + 128],
                                tri[:], op=OP.mult)
                        nc.tensor.matmul(
                            o_ps[:, lo:512], vhat[:, b, si, hh, :],
                            p_sb[:, off:off + w],
                            start=(si == 0), stop=(si == nsi - 1))
                        off += w
                oh = ohp.tile([65, 512], DT_A2A, tag="oh")
                nc.vector.tensor_copy(oh[0:64, :], o_ps[0:64, :])
                # row 64 ships the RECIPROCAL of the softmax denominator so
                # the receive side skips its dnm->f32->rcp->bf16 chain (it
                # sits on the post-collective critical path).  Partition-
                # ALIGNED scratch (row 64 -> row 64; the DVE cannot shift
                # partitions) and the native reciprocal instruction (the
                # custom-DVE approx op misbehaves on a PSUM source here)
                rcs = orp.tile([65, 512], F32, tag="rcs")
                nc.vector.reciprocal(rcs[64:65, :], o_ps[64:65, :])
                nc.vector.tensor_copy(oh[64:65, :], rcs[64:65, :])
                nc.sync.dma_start(a2a_in[hh][b * 4 + qb, :, :], oh[:])

            def half_prep(j, pe_bcast=False):
                """after AllToAll j: reciprocal of the raw denominators on
                the DVE (scalar engine untouched), broadcast across
                partitions, scale this head-half of o^T (rows j*64..+64).

                pe_bcast=True broadcasts via a K=1 PE matmul into the (now
                idle) attention psOC ring instead of gpsimd: hp0's
                broadcasts must not sit on the gpsimd queue, where they
                would either delay the second AllToAll's trigger or -- if
                placed after it -- block until the collective completes
                (collectives hold the gpsimd queue to completion)."""
                # row 64 of the payload already carries the reciprocal
                rd1 = cp.tile([1, NCORES, 512], BF16, tag=f"rd1{j}")
                nc.sync.dma_start(rd1[:], a2a_out[j][:, 64, :])
                for ft in range(NCT):
                    o_raw = orp.tile([64, 512], DT_A2A, tag="oraw")
                    nc.sync.dma_start(o_raw[:], a2a_out[j][ft, 0:64, :])
                    if pe_bcast:
                        bch = psOC.tile([64, 512], F32, tag="o",
                                        name=f"bch{j}_{ft}")
                        nc.tensor.matmul(bch[:], ones1[0:1, 0:64],
                                         rd1[0:1, ft, :],
                                         start=True, stop=True)
                    else:
                        bch = orp.tile([64, 512], BF16, tag="bch")
                        nc.gpsimd.partition_broadcast(bch[:], rd1[0:1, ft, :])
                    nc.vector.tensor_tensor(
                        oT[j * 64:(j + 1) * 64, ft, :], o_raw[:], bch[:],
                        op=OP.mult)

            # ---- phase 1+2 interleaved; A2A per head ----
            for b in range(B):
                for qb in range(NQB):
                    proj(b, qb)
                    attn(0, b, qb)
            nc.gpsimd.collective_compute(
                "AllToAll", OP.bypass, replica_groups=[list(range(NCORES))],
                ins=[a2a_in[0].opt()], outs=[a2a_out[0].opt()])
            for b in range(B):
                for qb in range(NQB):
                    attn(1, b, qb)
            # second AllToAll: the gpsimd queue is now exactly
            # [A2A#0][A2A#1] with nothing between, so this fires the moment
            # the attn(1,..) payload DMAs land.  The 0.210 pin places its
            # SIMULATED completion after the 0.200-pinned hp0/j0 work: the
            # scheduler strengthens semaphore waits to match its static
            # order, so work placed after a collective's simulated
            # completion gets chained to that collective at runtime.  The
            # pin is scheduling-only -- the real trigger still fires as
            # soon as the payload DMAs land.
            # keep the gpsimd PartitionBroadcast ucode warm for the head-1
            # normalization: this op sits on the gpsimd queue between the
            # collectives, so it runs right after AllToAll#0 completes and
            # does NOT delay the AllToAll#1 trigger
            gwarm = orp.tile([64, 16], BF16, tag="gwarm")
            nc.gpsimd.partition_broadcast(gwarm[:], ones1[0:1, 0:16])
            with tc.tile_wait_until(0.210):
                nc.gpsimd.collective_compute(
                    "AllToAll", OP.bypass,
                    replica_groups=[list(range(NCORES))],
                    ins=[a2a_in[1].opt()], outs=[a2a_out[1].opt()])

            # hp0 overlaps the second AllToAll.  tile_wait_until pins its
            # scheduler placement AFTER head-1 attention: without it the
            # list scheduler (whose collective model is optimistic)
            # interleaves it into the attn(1,..) engine streams, stalling
            # the in-order queues and delaying the AllToAll trigger.
            with tc.tile_wait_until(0.200):
                half_prep(0, pe_bcast=True)

            # attention PSUM pools -> 4x[128,1024] out-projection pool
            for c in reversed(psa_ctx):
                c.__exit__(None, None, None)
            ps4_ctx = tc.tile_pool(name="ps4", bufs=1, space="PSUM")
            ps4 = ps4_ctx.__enter__()

            y2s = [ps4.tile([128, 1024], F32, tag=f"y4_{tt}", name=f"y4_{tt}")
                   for tt in range(TS // 128)]

            def oproj(j, tts):
                """K rows j*64..j*64+64 of oT/wo for token tiles `tts`.
                ft-outer so consecutive matmuls hit different PSUM regions
                (same-region accumulation serializes the PE at ~2x cost).
                NOTE: full-K (128-row) matmuls spanning both AllToAll-
                written halves of oT are ~14us faster but produce
                intermittent wrong results (1-in-4 observed) -- the
                scheduler appears to under-synchronize one of the two
                writer sets; K64/K128 mixes deadlock outright.  Stay with
                the uniform split-K structure."""
                for ft in range(NCT):
                    for tt in tts:
                        for nb in range(2):
                            half = slice(nb * 512, (nb + 1) * 512)
                            nc.tensor.matmul(
                                y2s[tt][:, half],
                                oT[j * 64:(j + 1) * 64, ft,
                                   tt * 128:(tt + 1) * 128],
                                wo[j * 64:(j + 1) * 64, ft, half],
                                start=(j == 0 and ft == 0), stop=False)

            def ln_tail(tt):
                y2 = y2s[tt]
                yc = lnp.tile([128, C], BF16, tag="yc")
                s0 = lnp.tile([128, 1], F32, tag="s0")
                s1 = lnp.tile([128, 1], F32, tag="s1")
                q0 = lnp.tile([128, 1], F32, tag="q0")
                q1 = lnp.tile([128, 1], F32, tag="q1")
                for nb, (s_acc, q_acc) in enumerate(((s0, q0), (s1, q1))):
                    half = slice(nb * 512, (nb + 1) * 512)
                    yh = y2[:, half]
                    nc.tensor.matmul(yh, ones1[:], bo[:, half],
                                     start=False, stop=True)
                    # move to SBUF + row-sum on the scalar engine
                    nc.scalar.activation(yc[:, half], yh, AF.Copy,
                                         accum_out=s_acc[:])
                    # sum of squares on the scalar engine
                    sqh = lnp.tile([128, 512], BF16, tag=f"sqh{nb}")
                    nc.scalar.activation(sqh[:], yh, AF.Square,
                                         accum_out=q_acc[:])
                mu = lnp.tile([128, 1], F32, tag="mu")
                nc.vector.tensor_tensor(mu[:], s0[:], s1[:], op=OP.add)
                nc.vector.tensor_scalar_mul(mu[:], mu[:], 1.0 / C)
                var = lnp.tile([128, 1], F32, tag="var")
                nc.vector.tensor_tensor(var[:], q0[:], q1[:], op=OP.add)
                nc.vector.tensor_scalar_mul(var[:], var[:], 1.0 / C)
                m2 = lnp.tile([128, 1], F32, tag="m2")
                nc.vector.tensor_tensor(m2[:], mu[:], mu[:], op=OP.mult)
                nc.vector.tensor_tensor(var[:], var[:], m2[:], op=OP.subtract)
                # Sqrt shares its ACT table with Copy/Square (sqrt_and_others)
                # so the tail runs without any table switches
                sd = lnp.tile([128, 1], F32, tag="sd")
                nc.scalar.activation(sd[:], var[:], AF.Sqrt, bias=eps_t[:])
                istd = lnp.tile([128, 1], F32, tag="istd")
                nc.vector.reciprocal(istd[:], sd[:])
                yn = lnp.tile([128, C], BF16, tag="yn")
                nc.vector.tensor_scalar(
                    yn[:], yc[:], mu[:], istd[:], op0=OP.subtract, op1=OP.mult)
                yg = lnp.tile([128, C], BF16, tag="yg")
                nc.vector.tensor_tensor(yg[:], yn[:], gam[:], op=OP.mult)
                yf = lnp.tile([128, C], BF16, tag="yf")
                nc.vector.tensor_tensor(yf[:], yg[:], bet[:], op=OP.add)
                nc.sync.dma_start(out_h[tt * 128:(tt + 1) * 128, :], yf[:])

            # head-0 projection half, pinned with hp0 (same reasoning)
            with tc.tile_wait_until(0.200):
                oproj(0, [0, 1, 2, 3])

            half_prep(1)
            # head-1 half + LayerNorm, two token tiles at a time so the LN
            # of the first pair overlaps the second pair's matmuls
            oproj(1, [0, 1])
            ln_tail(0)
            ln_tail(1)
            oproj(1, [2, 3])
            ln_tail(2)
            ln_tail(3)
            ps4_ctx.__exit__(None, None, None)

    nc.compile()
    return nc


def _get_nc():
    if "nc" not in _CACHE:
        _CACHE["nc"] = _build()
    return _CACHE["nc"]


def _tile_w(w):
    m = w.shape[1]
    return np.ascontiguousarray(
        w.reshape(NCT, 128, m).transpose(1, 0, 2)).astype(NP_W)


def _make_in_maps(inputs):
    x = np.asarray(inputs["x"], np.float32)
    Wq = np.asarray(inputs["Wq"], np.float32)
    Wk = np.asarray(inputs["Wk"], np.float32)
    Wv = np.asarray(inputs["Wv"], np.float32)
    Wo = np.asarray(inputs["Wo"], np.float32)
    bq = np.asarray(inputs["bq"], np.float32)
    bk = np.asarray(inputs["bk"], np.float32)
    bv = np.asarray(inputs["bv"], np.float32)
    bo = np.asarray(inputs["bo"], np.float32)
    gamma = np.asarray(inputs["gamma"], np.float32)
    beta = np.asarray(inputs["beta"], np.float32)

    # [C, B, T] pre-tiled to [128, NCT, B, T] (partition-major)
    xT = np.ascontiguousarray(
        x.transpose(2, 0, 1).reshape(NCT, 128, B, T).transpose(1, 0, 2, 3)
    ).astype(NP_X)
    wo_c = np.ascontiguousarray(
        Wo.reshape(NCT, 128, C).transpose(1, 0, 2)).astype(NP_W)
    bo_row = np.ascontiguousarray(bo.reshape(1, C)).astype(ml_dtypes.bfloat16)
    gamb = np.ascontiguousarray(np.broadcast_to(gamma, (128, C))).astype(ml_dtypes.bfloat16)
    betb = np.ascontiguousarray(np.broadcast_to(beta, (128, C))).astype(ml_dtypes.bfloat16)

    maps = []
    for i in range(NCORES):
        cols = slice(DPC * i, DPC * (i + 1))
        maps.append({
            "xT": xT,
            "wq": _tile_w(Wq[:, cols]),
            "wk": _tile_w(Wk[:, cols]),
            "wv": _tile_w(Wv[:, cols]),
            "wo": wo_c,
            "bqT": np.ascontiguousarray(bq[cols].reshape(DPC, 1)),
            "bkT": np.ascontiguousarray(bk[cols].reshape(DPC, 1)),
            "bvT": np.ascontiguousarray(bv[cols].reshape(DPC, 1)),
            "bo_row": bo_row,
            "gamb": gamb,
            "betb": betb,
        })
    return maps


def _run(inputs, trace=False, **kwargs):
    nc = _get_nc()
    in_maps = _make_in_maps(inputs)
    res = run_bass_kernel_spmd(nc, in_maps, core_ids=list(range(NCORES)),
                               trace=trace, **kwargs)
    y = np.empty((B, T, C), np.float32)
    for i in range(NCORES):
        b, ts = divmod(i, 4)
        y[b, ts * TS:(ts + 1) * TS, :] = res.results[i]["out"].astype(np.float32)
    return y, res


def kernel(**inputs) -> np.ndarray:
    out, _ = _run(inputs, trace=False)
    return out



# revision 38
# speedup vs baseline: 1.2763x; 1.2763x over previous
"""Distributed Trainium2 Bass kernel for a causal attention block + LayerNorm.

Reference computation (B=2, T=2048, C=1024, H=16 heads, Dh=64):
    q,k,v = x@Wq+bq, x@Wk+bk, x@Wv+bv          (per-head split)
    att   = softmax(causal(q k^T / sqrt(Dh)))
    o     = att @ v ; y = o@Wo + bo ; out = LayerNorm(y) * gamma + beta

Sharding (8 cores, one TRN2 chip):
    Tensor-parallel over heads: core i owns heads {2i, 2i+1} for BOTH
    batches (Megatron-style column shards of Wq/Wk/Wv).  After attention,
    two 8-core AllToAlls (one per local head, bf16 payload) redistribute the
    per-head outputs (plus softmax denominators) to token-sharding: core i
    ends with tokens [b = i//4, t in (i%4)*512 ...] with ALL 1024 features,
    applies the softmax division, output projection (full Wo), bias and
    LayerNorm locally, and writes its (512, 1024) slice of the output.

Schedule (vs the v3 baseline, ~350us -> ~270us):
    - x^T DMA'd in 8 per-(b, q-block) slices in consumption order so the
      first projection starts ~8us in instead of waiting for the full 8.4MB;
      Wo + LN constants loaded at kernel start.
    - q/k projection accumulation chains interleaved (consecutive PE
      matmuls alternate PSUM banks; same-bank accumulation serializes the
      PE at ~2x cost) and attention emission software-pipelined: scores
      for group g+1 issued before P@V of group g.
    - softmax normalization entirely off the scalar engine: raw
      denominators ride the AllToAll (row 64); receive side does bf16->
      fp32 copy -> DVE reciprocal_approx_fast -> bf16, a partition
      broadcast (K=1 PE matmul into the idle psOC ring for hp0, gpsimd
      for hp1), multiply on DVE.  The scalar queue holds ONLY exps + the
      LayerNorm accumulations, in two stable activation tables, so no
      per-tile ACT_TABLE_LOADs and no cross-collective scalar dependency
      a hoisted schedule could head-of-line block on (v3 lost ~22us).
    - second AllToAll triggered the moment head-1 attention's payload
      DMAs land: the gpsimd queue is exactly [A2A#0][A2A#1] (collectives
      hold that queue to completion, so nothing may sit between).
    - output projection split-K: the head-0 half (K rows 0..63 of oT/Wo,
      all 4 token tiles) accumulates into PSUM while the second AllToAll
      is in flight; the head-1 half + bias + LayerNorm land after it, two
      token tiles at a time, ft-outer so consecutive matmuls alternate
      PSUM regions.  Attention PSUM pools are closed and a 4x[128,1024]
      pool opened so all 4 token tiles accumulate concurrently.
    - scheduler control: the overlap work carries tile_wait_until(0.200)
      and the second AllToAll tile_wait_until(0.210).  The list scheduler
      strengthens semaphore waits to match its simulated order, so (a)
      un-pinned overlap work gets interleaved into the attn(1,..) engine
      queues (its collective model is optimistic), and (b) work placed
      after a collective's simulated completion gets chained to that
      collective at runtime.  Pinning the trigger LATER than its overlap
      work is what keeps the overlap waits encoded against A2A#0 only;
      the pins are scheduling-time-only and cost nothing at runtime.

Layout choices (all on-chip matmuls contract over the partition axis):
    - activations are feature-major: host passes x^T [C, B, T].
    - q^T,k^T,v^T [d, t] produced directly; v transposed on the PE into
      s-major v-hat [s, d] with an extra ones column per head so the P@V
      matmul also yields the softmax denominator for free.
    - scores are computed transposed: S^T[s, q] = k^T.T @ q^T; score chunks
      are packed in pairs into 2-bank PSUM tiles so each scalar-engine Exp
      call covers up to 1024 columns; causal masking via a triangular
      bf16 multiply on the DVE for the 128-wide diagonal blocks only;
      P@V uses v-hat as the stationary operand so the unnormalized attention
      output O^T [d, q] is produced feature-major (no transposes needed).
"""

import numpy as np
import ml_dtypes

import concourse.bass as bass
import concourse.mybir as mybir
import concourse.tile as tile
from concourse import bacc
from concourse.bass_utils import run_bass_kernel_spmd
F32 = mybir.dt.float32
BF16 = mybir.dt.bfloat16
AF = mybir.ActivationFunctionType
OP = mybir.AluOpType

B, T, C, H, Dh = 2, 2048, 1024, 16, 64
NCORES = 8
HPC = 2               # heads per core
DPC = HPC * Dh        # 128 feature columns per core
TS = 512              # output token-slice length per core
NQB = T // 512        # 4 q blocks
NST = T // 128        # 16 s tiles
NCT = C // 128        # 8 contraction tiles
EPS = 1e-5

DT_X = BF16
DT_W = BF16
DT_P = BF16
DT_A2A = BF16         # AllToAll payload dtype
NP_X = ml_dtypes.bfloat16
NP_W = ml_dtypes.bfloat16

_CACHE = {}


def _build():
    nc = bacc.Bacc("TRN2", target_bir_lowering=False, debug=False,
                   num_devices=NCORES)

    xT_h = nc.dram_tensor("xT", [128, NCT, B, T], DT_X, kind="ExternalInput")
    wq_h = nc.dram_tensor("wq", [128, NCT, DPC], DT_W, kind="ExternalInput")
    wk_h = nc.dram_tensor("wk", [128, NCT, DPC], DT_W, kind="ExternalInput")
    wv_h = nc.dram_tensor("wv", [128, NCT, DPC], DT_W, kind="ExternalInput")
    wo_h = nc.dram_tensor("wo", [128, NCT, C], DT_W, kind="ExternalInput")
    bqT_h = nc.dram_tensor("bqT", [DPC, 1], F32, kind="ExternalInput")
    bkT_h = nc.dram_tensor("bkT", [DPC, 1], F32, kind="ExternalInput")
    bvT_h = nc.dram_tensor("bvT", [DPC, 1], F32, kind="ExternalInput")
    bo_h = nc.dram_tensor("bo_row", [1, C], BF16, kind="ExternalInput")
    gam_h = nc.dram_tensor("gamb", [128, C], BF16, kind="ExternalInput")
    bet_h = nc.dram_tensor("betb", [128, C], BF16, kind="ExternalInput")
    out_h = nc.dram_tensor("out", [TS, C], BF16, kind="ExternalOutput")

    ones1_d = nc.inline_tensor(np.ones((1, 128), ml_dtypes.bfloat16), name="ones1_const")
    ident_d = nc.inline_tensor(
        np.eye(128, dtype=ml_dtypes.bfloat16), name="ident_const")
    tri_np = (np.tril(np.ones((128, 128), np.float32)).T).astype(ml_dtypes.bfloat16)
    tri_d = nc.inline_tensor(tri_np, name="tri_const")

    with tile.TileContext(nc) as tc:
        with (
            tc.tile_pool(name="const", bufs=1) as cp,
            tc.tile_pool(name="dram", bufs=1, space="DRAM") as dp,
            tc.tile_pool(name="act", bufs=1) as ap,
            tc.tile_pool(name="xw", bufs=1) as xw,
            tc.tile_pool(name="wop", bufs=1) as wop,
            tc.tile_pool(name="lnp", bufs=2) as lnp,
            tc.tile_pool(name="pp", bufs=5) as pp,
            tc.tile_pool(name="vtp", bufs=2) as vtp,
            tc.tile_pool(name="ohp", bufs=3) as ohp,
            tc.tile_pool(name="orp", bufs=3) as orp,
        ):
            # attention-phase PSUM pools (closed before the out-projection,
            # which needs all 8 banks for 4 concurrent [128,1024] tiles)
            psa_ctx = [
                tc.tile_pool(name="psM", bufs=2, space="PSUM"),
                tc.tile_pool(name="psS2", bufs=2, space="PSUM"),
                tc.tile_pool(name="psOC", bufs=2, space="PSUM"),
            ]
            psM, psS2, psOC = (c.__enter__() for c in psa_ctx)
            psM, psS2, psOC = psM, psS2, psOC

            # ---- weights first (small, needed immediately); x^T in 32
            # per-(b,qb) slices on the gpsimd queue, in the exact order the
            # projections consume them ----
            wq = xw.tile([128, NCT, DPC], DT_W)
            wk = xw.tile([128, NCT, DPC], DT_W)
            wv = xw.tile([128, NCT, DPC], DT_W)
            for w_sb, w_h in ((wq, wq_h), (wk, wk_h), (wv, wv_h)):
                nc.sync.dma_start(w_sb[:], w_h[:])

            # warm-up AllToAll FIRST on the gpsimd queue: the runtime's
            # pre-first-collective barrier (40-128us, cross-core variance)
            # runs concurrently with phase 1 either way, but the warmup's
            # trigger must not queue behind the DMA-issue backlog -- on
            # slow-barrier runs that serialized straight into AllToAll#0
            warm_in = dp.tile([NCORES, 8], F32, tag="wi")
            warm_out = dp.tile([NCORES, 8], F32, tag="wo_")
            warm_sb = cp.tile([NCORES, 8], F32)
            nc.gpsimd.memset(warm_sb[:], 0.0)
            nc.sync.dma_start(warm_in[:], warm_sb[:])
            nc.gpsimd.collective_compute(
                "AllToAll", OP.bypass, replica_groups=[list(range(NCORES))],
                ins=[warm_in.opt()], outs=[warm_out.opt()])

            xT = xw.tile([128, NCT, B, T], DT_X)
            for b in range(B):
                for qb in range(NQB):
                    sl = slice(qb * 512, (qb + 1) * 512)
                    # one DMA per slice: each gpsimd dma_start issue costs
                    # ~640ns of DIRECT2D, so ct-chunking the first slice
                    # (8 issues) DELAYS its completion past a single issue
                    nc.gpsimd.dma_start(xT[:, :, b, sl],
                                        xT_h[:, :, b, sl])
            # Wo + LN constants now -- they are needed right after the last
            # collective and must not queue behind it
            wo = wop.tile([128, NCT, C], DT_W, tag="wo")
            nc.gpsimd.dma_start(wo[:], wo_h[:])
            bo = wop.tile([1, C], BF16, tag="bo")
            nc.sync.dma_start(bo[:], bo_h[:])
            gam = wop.tile([128, C], BF16, tag="gam")
            nc.sync.dma_start(gam[:], gam_h[:])
            bet = wop.tile([128, C], BF16, tag="bet")
            nc.sync.dma_start(bet[:], bet_h[:])
            eps_t = wop.tile([128, 1], F32, tag="eps")
            nc.gpsimd.memset(eps_t[:], EPS)

            # ---- small constants (issued on sync queue) ----
            bqT = cp.tile([DPC, 1], F32)
            nc.sync.dma_start(bqT[:], bqT_h[:])
            bkT = cp.tile([DPC, 1], F32)
            nc.sync.dma_start(bkT[:], bkT_h[:])
            bvT = cp.tile([DPC, 1], F32)
            nc.sync.dma_start(bvT[:], bvT_h[:])
            ident = cp.tile([128, 128], BF16)
            nc.sync.dma_start(ident[:], ident_d[:])
            ones1 = cp.tile([1, 128], BF16)
            nc.sync.dma_start(ones1[:], ones1_d[:])
            tri = cp.tile([128, 128], BF16)
            nc.sync.dma_start(tri[:], tri_d[:])

            # ---- persistent activation tiles ----
            qT = ap.tile([DPC, B, T], DT_P)
            kT = ap.tile([DPC, B, T], DT_P)
            vhat = ap.tile([128, B, NST, HPC, 65], DT_P)
            oT = ap.tile([128, NCT, 512], DT_P)
            for b in range(B):
                nc.gpsimd.memset(vhat[:, b, :, :, 64:65], 1.0)

            a2a_in0 = dp.tile([NCORES, 65, 512], DT_A2A, tag="ai0")
            a2a_in1 = dp.tile([NCORES, 65, 512], DT_A2A, tag="ai1")
            a2a_out0 = dp.tile([NCORES, 65, 512], DT_A2A, tag="ao0")
            a2a_out1 = dp.tile([NCORES, 65, 512], DT_A2A, tag="ao1")
            a2a_in = [a2a_in0, a2a_in1]
            a2a_out = [a2a_out0, a2a_out1]

            def proj(b, qb):
                sl = slice(qb * 512, (qb + 1) * 512)
                # q and k accumulation chains interleaved: consecutive PE
                # matmuls alternate PSUM banks (same-bank accumulation
                # serializes the PE at ~2x cost)
                ps_q = psM.tile([128, 512], F32, tag="m", name="ps_q")
                ps_k = psM.tile([128, 512], F32, tag="m", name="ps_k")
                for ct in range(NCT):
                    nc.tensor.matmul(ps_q[:], wq[:, ct], xT[:, ct, b, sl],
                                     start=(ct == 0), stop=(ct == NCT - 1))
                    nc.tensor.matmul(ps_k[:], wk[:, ct], xT[:, ct, b, sl],
                                     start=(ct == 0), stop=(ct == NCT - 1))
                nc.vector.tensor_scalar_add(qT[:, b, sl], ps_q[:], bqT[:])
                nc.vector.tensor_scalar_add(kT[:, b, sl], ps_k[:], bkT[:])
                # v^T, then transpose 128x128 blocks into s-major vhat
                ps = psM.tile([128, 512], F32, tag="m")
                for ct in range(NCT):
                    nc.tensor.matmul(ps[:], wv[:, ct], xT[:, ct, b, sl],
                                     start=(ct == 0), stop=(ct == NCT - 1))
                vt = vtp.tile([128, 512], DT_P, tag="vt")
                nc.vector.tensor_scalar_add(vt[:], ps[:], bvT[:])
                for sub in range(4):
                    st = qb * 4 + sub
                    tr = psM.tile([128, 128], DT_P, tag="m")
                    nc.tensor.transpose(
                        tr[:], vt[:, sub * 128:(sub + 1) * 128], ident[:])
                    nc.vector.tensor_copy(
                        vhat[:, b, st, :, 0:64],
                        tr[:].rearrange("p (hh d) -> p hh d", hh=HPC))

            def attn(hh, b, qb):
                hlo = hh * 64
                o_ps = psOC.tile([65, 512], F32, tag="o")
                nsi = 4 * qb + 4
                # chunks (si, lo): lo = in-block column offset; pack pairs
                # into one 2-bank PSUM tile so each exp covers both
                chunks = [(si, 0) for si in range(4 * qb)] + \
                         [(si, si * 128 - qb * 512) for si in range(4 * qb, nsi)]
                groups = []
                i = 0
                while i < len(chunks):
                    w0 = 512 - chunks[i][1]
                    if i + 1 < len(chunks) and w0 + (512 - chunks[i + 1][1]) <= 1024:
                        groups.append([chunks[i], chunks[i + 1]])
                        i += 2
                    else:
                        groups.append([chunks[i]])
                        i += 1

                def emit_scores(grp):
                    s_ps = psS2.tile([128, 1024], F32, tag="s2")
                    off = 0
                    for si, lo in grp:
                        w = 512 - lo
                        nc.tensor.matmul(
                            s_ps[:, off:off + w],
                            kT[hlo:hlo + 64, b, si * 128:(si + 1) * 128],
                            qT[hlo:hlo + 64, b, qb * 512 + lo:(qb + 1) * 512],
                            start=True, stop=True)
                        off += w
                    return s_ps

                # software pipeline: scores for group g+1 are on the PE queue
                # before P@V of group g, so the PE never waits out the exp
                s_cur = emit_scores(groups[0])
                for g, grp in enumerate(groups):
                    tot = sum(512 - lo for _, lo in grp)
                    p_sb = pp.tile([128, 1024], DT_P, tag="p")
                    nc.scalar.activation(p_sb[:, 0:tot], s_cur[:, 0:tot],
                                         AF.Exp, scale=0.125)
                    if g + 1 < len(groups):
                        s_cur = emit_scores(groups[g + 1])
                    off = 0
                    for si, lo in grp:
                        w = 512 - lo
                        if lo > 0 or si * 128 == qb * 512:
                            # diagonal block: causal triangle mask (on DVE --
                            # gpsimd hosts the collective triggers and would
                            # serialize behind them)
                            nc.vector.tensor_tensor(
                                p_sb[:, off:off + 128], p_sb[:, off:off + 128],
                                tri[:], op=OP.mult)
                        nc.tensor.matmul(
                            o_ps[:, lo:512], vhat[:, b, si, hh, :],
                            p_sb[:, off:off + w],
                            start=(si == 0), stop=(si == nsi - 1))
                        off += w
                oh = ohp.tile([65, 512], DT_A2A, tag="oh")
                nc.vector.tensor_copy(oh[:], o_ps[:])
                nc.sync.dma_start(a2a_in[hh][b * 4 + qb, :, :], oh[:])

            def half_prep(j, pe_bcast=False):
                """after AllToAll j: reciprocal of the raw denominators on
                the DVE (scalar engine untouched), broadcast across
                partitions, scale this head-half of o^T (rows j*64..+64).

                pe_bcast=True broadcasts via a K=1 PE matmul into the (now
                idle) attention psOC ring instead of gpsimd: hp0's
                broadcasts must not sit on the gpsimd queue, where they
                would either delay the second AllToAll's trigger or -- if
                placed after it -- block until the collective completes
                (collectives hold the gpsimd queue to completion)."""
                dnm = cp.tile([NCORES, 512], DT_A2A, tag=f"dnm{j}")
                nc.sync.dma_start(dnm[:], a2a_out[j][:, 64, :])
                dnf = cp.tile([NCORES, 512], F32, tag=f"dnf{j}")
                nc.vector.tensor_copy(dnf[:], dnm[:])
                rcp = cp.tile([NCORES, 512], F32, tag=f"rcp{j}")
                nc.vector.reciprocal_approx_fast(rcp[:], dnf[:])
                rde = cp.tile([NCORES, 512], BF16, tag=f"rde{j}")
                nc.vector.tensor_copy(rde[:], rcp[:])
                rd1 = cp.tile([1, NCORES, 512], BF16, tag=f"rd1{j}")
                nc.sync.dma_start(rd1[:], rde[:])
                for ft in range(NCT):
                    o_raw = orp.tile([64, 512], DT_A2A, tag="oraw")
                    nc.sync.dma_start(o_raw[:], a2a_out[j][ft, 0:64, :])
                    if pe_bcast:
                        bch = psOC.tile([64, 512], F32, tag="o",
                                        name=f"bch{j}_{ft}")
                        nc.tensor.matmul(bch[:], ones1[0:1, 0:64],
                                         rd1[0:1, ft, :],
                                         start=True, stop=True)
                    else:
                        bch = orp.tile([64, 512], BF16, tag="bch")
                        nc.gpsimd.partition_broadcast(bch[:], rd1[0:1, ft, :])
                    nc.vector.tensor_tensor(
                        oT[j * 64:(j + 1) * 64, ft, :], o_raw[:], bch[:],
                        op=OP.mult)

            # ---- phase 1+2 interleaved; A2A per head ----
            for b in range(B):
                for qb in range(NQB):
                    proj(b, qb)
                    attn(0, b, qb)
            nc.gpsimd.collective_compute(
                "AllToAll", OP.bypass, replica_groups=[list(range(NCORES))],
                ins=[a2a_in[0].opt()], outs=[a2a_out[0].opt()])
            for b in range(B):
                for qb in range(NQB):
                    attn(1, b, qb)
            # second AllToAll: the gpsimd queue is now exactly
            # [A2A#0][A2A#1] with nothing between, so this fires the moment
            # the attn(1,..) payload DMAs land.  The 0.210 pin places its
            # SIMULATED completion after the 0.200-pinned hp0/j0 work: the
            # scheduler strengthens semaphore waits to match its static
            # order, so work placed after a collective's simulated
            # completion gets chained to that collective at runtime.  The
            # pin is scheduling-only -- the real trigger still fires as
            # soon as the payload DMAs land.
            with tc.tile_wait_until(0.210):
                nc.gpsimd.collective_compute(
                    "AllToAll", OP.bypass,
                    replica_groups=[list(range(NCORES))],
                    ins=[a2a_in[1].opt()], outs=[a2a_out[1].opt()])

            # hp0 overlaps the second AllToAll.  tile_wait_until pins its
            # scheduler placement AFTER head-1 attention: without it the
            # list scheduler (whose collective model is optimistic)
            # interleaves it into the attn(1,..) engine streams, stalling
            # the in-order queues and delaying the AllToAll trigger.
            with tc.tile_wait_until(0.200):
                half_prep(0, pe_bcast=True)

            # attention PSUM pools -> 4x[128,1024] out-projection pool
            for c in reversed(psa_ctx):
                c.__exit__(None, None, None)
            ps4_ctx = tc.tile_pool(name="ps4", bufs=1, space="PSUM")
            ps4 = ps4_ctx.__enter__()

            y2s = [ps4.tile([128, 1024], F32, tag=f"y4_{tt}", name=f"y4_{tt}")
                   for tt in range(TS // 128)]

            def oproj(j, tts):
                """K rows j*64..j*64+64 of oT/wo for token tiles `tts`.
                ft-outer so consecutive matmuls hit different PSUM regions
                (same-region accumulation serializes the PE at ~2x cost).
                NOTE: full-K (128-row) matmuls spanning both AllToAll-
                written halves of oT are ~14us faster but produce
                intermittent wrong results (1-in-4 observed) -- the
                scheduler appears to under-synchronize one of the two
                writer sets; K64/K128 mixes deadlock outright.  Stay with
                the uniform split-K structure."""
                for ft in range(NCT):
                    for tt in tts:
                        for nb in range(2):
                            half = slice(nb * 512, (nb + 1) * 512)
                            nc.tensor.matmul(
                                y2s[tt][:, half],
                                oT[j * 64:(j + 1) * 64, ft,
                                   tt * 128:(tt + 1) * 128],
                                wo[j * 64:(j + 1) * 64, ft, half],
                                start=(j == 0 and ft == 0), stop=False)

            def ln_tail(tt):
                y2 = y2s[tt]
                yc = lnp.tile([128, C], BF16, tag="yc")
                s0 = lnp.tile([128, 1], F32, tag="s0")
                s1 = lnp.tile([128, 1], F32, tag="s1")
                q0 = lnp.tile([128, 1], F32, tag="q0")
                q1 = lnp.tile([128, 1], F32, tag="q1")
                for nb, (s_acc, q_acc) in enumerate(((s0, q0), (s1, q1))):
                    half = slice(nb * 512, (nb + 1) * 512)
                    yh = y2[:, half]
                    nc.tensor.matmul(yh, ones1[:], bo[:, half],
                                     start=False, stop=True)
                    # move to SBUF + row-sum on the scalar engine
                    nc.scalar.activation(yc[:, half], yh, AF.Copy,
                                         accum_out=s_acc[:])
                    # sum of squares on the scalar engine
                    sqh = lnp.tile([128, 512], BF16, tag=f"sqh{nb}")
                    nc.scalar.activation(sqh[:], yh, AF.Square,
                                         accum_out=q_acc[:])
                mu = lnp.tile([128, 1], F32, tag="mu")
                nc.vector.tensor_tensor(mu[:], s0[:], s1[:], op=OP.add)
                nc.vector.tensor_scalar_mul(mu[:], mu[:], 1.0 / C)
                var = lnp.tile([128, 1], F32, tag="var")
                nc.vector.tensor_tensor(var[:], q0[:], q1[:], op=OP.add)
                nc.vector.tensor_scalar_mul(var[:], var[:], 1.0 / C)
                m2 = lnp.tile([128, 1], F32, tag="m2")
                nc.vector.tensor_tensor(m2[:], mu[:], mu[:], op=OP.mult)
                nc.vector.tensor_tensor(var[:], var[:], m2[:], op=OP.subtract)
                # Sqrt shares its ACT table with Copy/Square (sqrt_and_others)
                # so the tail runs without any table switches
                sd = lnp.tile([128, 1], F32, tag="sd")
                nc.scalar.activation(sd[:], var[:], AF.Sqrt, bias=eps_t[:])
                istd = lnp.tile([128, 1], F32, tag="istd")
                nc.vector.reciprocal(istd[:], sd[:])
                yn = lnp.tile([128, C], BF16, tag="yn")
                nc.vector.tensor_scalar(
                    yn[:], yc[:], mu[:], istd[:], op0=OP.subtract, op1=OP.mult)
                yg = lnp.tile([128, C], BF16, tag="yg")
                nc.vector.tensor_tensor(yg[:], yn[:], gam[:], op=OP.mult)
                yf = lnp.tile([128, C], BF16, tag="yf")
                nc.vector.tensor_tensor(yf[:], yg[:], bet[:], op=OP.add)
                nc.sync.dma_start(out_h[tt * 128:(tt + 1) * 128, :], yf[:])

            # head-0 projection half, pinned with hp0 (same reasoning)
            with tc.tile_wait_until(0.200):
                oproj(0, [0, 1, 2, 3])

            half_prep(1)
            # head-1 half + LayerNorm, two token tiles at a time so the LN
            # of the first pair overlaps the second pair's matmuls
            oproj(1, [0, 1])
            ln_tail(0)
            ln_tail(1)
            oproj(1, [2, 3])
            ln_tail(2)
            ln_tail(3)
            ps4_ctx.__exit__(None, None, None)

    nc.compile()
    return nc


def _get_nc():
    if "nc" not in _CACHE:
        _CACHE["nc"] = _build()
    return _CACHE["nc"]


def _tile_w(w):
    m = w.shape[1]
    return np.ascontiguousarray(
        w.reshape(NCT, 128, m).transpose(1, 0, 2)).astype(NP_W)


def _make_in_maps(inputs):
    x = np.asarray(inputs["x"], np.float32)
    Wq = np.asarray(inputs["Wq"], np.float32)
    Wk = np.asarray(inputs["Wk"], np.float32)
    Wv = np.asarray(inputs["Wv"], np.float32)
    Wo = np.asarray(inputs["Wo"], np.float32)
    bq = np.asarray(inputs["bq"], np.float32)
    bk = np.asarray(inputs["bk"], np.float32)
    bv = np.asarray(inputs["bv"], np.float32)
    bo = np.asarray(inputs["bo"], np.float32)
    gamma = np.asarray(inputs["gamma"], np.float32)
    beta = np.asarray(inputs["beta"], np.float32)

    # [C, B, T] pre-tiled to [128, NCT, B, T] (partition-major)
    xT = np.ascontiguousarray(
        x.transpose(2, 0, 1).reshape(NCT, 128, B, T).transpose(1, 0, 2, 3)
    ).astype(NP_X)
    wo_c = np.ascontiguousarray(
        Wo.reshape(NCT, 128, C).transpose(1, 0, 2)).astype(NP_W)
    bo_row = np.ascontiguousarray(bo.reshape(1, C)).astype(ml_dtypes.bfloat16)
    gamb = np.ascontiguousarray(np.broadcast_to(gamma, (128, C))).astype(ml_dtypes.bfloat16)
    betb = np.ascontiguousarray(np.broadcast_to(beta, (128, C))).astype(ml_dtypes.bfloat16)

    maps = []
    for i in range(NCORES):
        cols = slice(DPC * i, DPC * (i + 1))
        maps.append({
            "xT": xT,
            "wq": _tile_w(Wq[:, cols]),
            "wk": _tile_w(Wk[:, cols]),
            "wv": _tile_w(Wv[:, cols]),
            "wo": wo_c,
            "bqT": np.ascontiguousarray(bq[cols].reshape(DPC, 1)),
            "bkT": np.ascontiguousarray(bk[cols].reshape(DPC, 1)),
            "bvT": np.ascontiguousarray(bv[cols].reshape(DPC, 1)),
            "bo_row": bo_row,
            "gamb": gamb,
            "betb": betb,
        })
    return maps


def _run(inputs, trace=False, **kwargs):
    nc = _get_nc()
    in_maps = _make_in_maps(inputs)
    res = run_bass_kernel_spmd(nc, in_maps, core_ids=list(range(NCORES)),
                               trace=trace, **kwargs)
    y = np.empty((B, T, C), np.float32)
    for i in range(NCORES):
        b, ts = divmod(i, 4)
        y[b, ts * TS:(ts + 1) * TS, :] = res.results[i]["out"].astype(np.float32)
    return y, res


def kernel(**inputs) -> np.ndarray:
    out, _ = _run(inputs, trace=False)
    return out

